# revision 22
# baseline (speedup 1.0000x reference)
"""Causal multi-head self-attention with RoPE on 8 TRN2 NeuronCores.

Sharding: data-parallel over batch (2) x tensor-parallel over heads (4 groups
of 4 heads).  Core c handles batch c//4, head group c%4.  Q/K/V projections and
attention are fully local per core; the output projection is partial over the
local 256 head-dims and finished with a ReduceScatter over each batch's 4-core
group, chunked by query block so the collective overlaps attention compute.

Device kernel design notes (v2, bf16):
 - All matmul operands are bf16 (psum accumulation fp32): same PE streaming
   rate as float32r but half the LDWEIGHTS time, half the DMA bytes and half
   the SBUF footprint.  Measured rel-err vs the fp32 reference ~1e-2 budget.
 - Weights / x / tables are host-packed so every load is one DMA with
   multi-KB contiguous runs per partition (the v1 per-chunk loads produced
   thousands of sub-1KB descriptors).
 - DMA issue order = first-use order (wq, wk, x half 0, rope tables, wv, ...)
   so the first projection matmul starts as early as possible.
 - Scores are computed transposed (keys on partitions, queries on the free
   axis) so softmax normalization needs only a partition-broadcast of the
   reciprocal denominator.
 - Softmax is unnormalized-exp without max subtraction (scores are ~N(0,1));
   the denominator comes from a ones column appended to V (matmul row 64
   accumulates sum_k exp).  1/den via the fast DVE reciprocal approximation.
 - RoPE: Q/K weight rows are pre-permuted on host (per head: even dims then
   odd dims) so the device rotation works on contiguous 32-row blocks; the
   permutation cancels in the QK^T contraction.
"""

import os
import sys

for _p in ("/opt/trn_rl_repo",):
    if os.path.isdir(_p) and _p not in sys.path:
        sys.path.insert(0, _p)

import numpy as np
import ml_dtypes

import concourse.bass as bass
import concourse.mybir as mybir
from concourse.bacc import Bacc
from concourse.tile import TileContext
from concourse.bass_utils import run_bass_kernel_spmd

D = 1024          # model dim
H = 16            # heads
DK = 64           # head dim
B = 2             # batch
L = 2048          # sequence
HPG = 4           # heads per group (per core)
DG = HPG * DK     # 256 local head dims
QB = 512          # query block (matmul free dim)
NQB = L // QB     # 4 query blocks
KT = 128          # key tile (psum partition dim)
LH = L // 2       # sequence half processed per projection pass
THETA = 10000.0

F32 = mybir.dt.float32
BF16 = mybir.dt.bfloat16

REPLICA_GROUPS = [[0, 1, 2, 3], [4, 5, 6, 7]]

EC = D // 128  # 8 contraction chunks over the model dim
NKT = L // KT  # 16 key tiles


def build_graph() -> bass.Bass:
    nc = Bacc(num_devices=8)

    # host-packed layouts: partition-major so each load is one DMA with
    # contiguous multi-KB runs per partition
    xh = nc.declare_dram_parameter("xh", [128, 4, EC, QB], BF16, isOutput=False)
    wq = nc.declare_dram_parameter("wq", [128, EC, DG], BF16, isOutput=False)
    wk = nc.declare_dram_parameter("wk", [128, EC, DG], BF16, isOutput=False)
    wv = nc.declare_dram_parameter("wv", [128, EC, DG], BF16, isOutput=False)
    wo = nc.declare_dram_parameter("wo", [128, 2, D], BF16, isOutput=False)
    cs = nc.declare_dram_parameter("cs", [128, L], F32, isOutput=False)
    sn = nc.declare_dram_parameter("sn", [128, L], F32, isOutput=False)
    tri = nc.declare_dram_parameter("tri", [KT, KT], BF16, isOutput=False)
    out_ext = nc.declare_dram_parameter("out", [QB, D], BF16, isOutput=True)

    DKP = DK + 2  # V row padded to 66 bf16 (132B): keeps every per-head
    #               column 4-byte aligned (hw APs are word-addressed);
    #               col 64 = ones (softmax denominator), col 65 = 0

    with TileContext(nc) as tc:
        with (
            tc.tile_pool(name="const", bufs=1) as cpool,
            tc.tile_pool(name="work", bufs=2) as wpool,
            tc.tile_pool(name="ps", bufs=2, space="PSUM") as pspool,
            tc.tile_pool(name="dram", bufs=1, space="DRAM") as dpool,
        ):
            # ---------------- constants / weights, in first-use order -------
            wq_sb = cpool.tile([128, EC, DG], BF16)
            wk_sb = cpool.tile([128, EC, DG], BF16)
            x_sb = [
                cpool.tile([128, EC, QB], BF16, name=f"x_sb{q}", tag=f"x{q}")
                for q in range(4)
            ]
            nc.sync.dma_start(out=wq_sb[:], in_=wq[:])
            nc.sync.dma_start(out=x_sb[0][:], in_=xh[:, 0])
            nc.sync.dma_start(out=wk_sb[:], in_=wk[:])

            cs_sb = cpool.tile([128, L], F32)
            sn_sb = cpool.tile([128, L], F32)
            nc.sync.dma_start(out=cs_sb[:], in_=cs[:])
            nc.sync.dma_start(out=sn_sb[:], in_=sn[:])
            nc.sync.dma_start(out=x_sb[1][:], in_=xh[:, 1])

            wv_sb = cpool.tile([128, EC, DG], BF16)
            nc.sync.dma_start(out=wv_sb[:], in_=wv[:])
            tri_sb = cpool.tile([KT, KT], BF16)
            nc.sync.dma_start(out=tri_sb[:], in_=tri[:])

            nc.sync.dma_start(out=x_sb[2][:], in_=xh[:, 2])
            nc.sync.dma_start(out=x_sb[3][:], in_=xh[:, 3])
            wo_sb = cpool.tile([128, 2, D], BF16)
            nc.sync.dma_start(out=wo_sb[:], in_=wo[:])

            # V with an appended ones column per (key tile, head): row 64 of
            # the attention matmul then accumulates the softmax denominator.
            v_aug = cpool.tile([128, NKT, HPG, DKP], BF16)
            nc.vector.memset(v_aug[:, :, :, DK:DK + 1], 1.0)
            nc.vector.memset(v_aug[:, :, :, DK + 1:DKP], 0.0)

            qt_sb = cpool.tile([128, 2, L], BF16)   # roped Q^T  (d on partitions)
            kt_sb = cpool.tile([128, 2, L], BF16)   # roped K^T
            ot_sb = cpool.tile([128, 2, L], BF16)   # normalized attention out^T



            # ---------------- projections + rope, per 512-col quarter ------
            for qtr in range(4):
                xts = x_sb[qtr]
                cols = slice(qtr * QB, (qtr + 1) * QB)

                # Q^T / K^T : [d-chunk 128, l 512] tiles, rope on eviction
                for w_sb, dst in ((wq_sb, qt_sb), (wk_sb, kt_sb)):
                    for ch in range(2):
                        ps = pspool.tile([128, QB], F32, name="ps_proj",
                                         tag="pp", bufs=3)
                        for e in range(EC):
                            nc.tensor.matmul(
                                ps[:],
                                w_sb[:, e, ch * 128:(ch + 1) * 128],
                                xts[:, e, :],
                                start=(e == 0),
                                stop=(e == EC - 1),
                            )
                        # rope: per 64-row head block [E(32); O(32)]:
                        #   E' = E*cos - O*sin ; O' = O*cos + E*sin
                        # sn carries the sign (E rows +sin, O rows -sin);
                        # the sin product is written partition-swapped so
                        # the final add is one full-width same-partition op
                        # (walrus requires tensor_tensor INPUTS to share
                        # partition ranges; outputs may shift).
                        t_ro = wpool.tile([128, QB], F32, name="t_ro", tag="t_ro")
                        u_ro = wpool.tile([128, QB], F32, name="u_ro", tag="u_ro")
                        nc.vector.tensor_mul(t_ro[:], ps[:], cs_sb[:, cols])
                        for p0 in (0, 64):
                            nc.vector.tensor_mul(
                                u_ro[p0:p0 + 32, :],
                                ps[p0 + 32:p0 + 64, :],
                                sn_sb[p0 + 32:p0 + 64, cols],
                            )
                            nc.vector.tensor_mul(
                                u_ro[p0 + 32:p0 + 64, :],
                                ps[p0:p0 + 32, :],
                                sn_sb[p0:p0 + 32, cols],
                            )
                        nc.vector.tensor_add(dst[:, ch, cols], t_ro[:], u_ro[:])

                # V : natural [l 128, d 256] tiles
                for v_lt in range(4):
                    lt = qtr * 4 + v_lt
                    psv = pspool.tile([128, DG], F32, name="ps_v",
                                      tag="pv", bufs=1)
                    for e in range(EC):
                        nc.tensor.matmul(
                            psv[:],
                            xts[:, e, v_lt * 128:(v_lt + 1) * 128],
                            wv_sb[:, e, :],
                            start=(e == 0),
                            stop=(e == EC - 1),
                        )
                    nc.vector.tensor_copy(
                        v_aug[:, lt, :, 0:DK],
                        psv.rearrange("p (h d) -> p h d", h=HPG),
                    )

            # ---------------- attention + output projection ----------------
            for qb in range(NQB):
                nkt = (qb + 1) * (QB // KT)  # causal: key tiles 0..nkt-1
                # unnormalized AV per head (bf16 SBUF copy releases the psum
                # bank fast); denominators batched in den4 so the slow DVE
                # reciprocal runs once per qb
                u_avs = []
                for h in range(HPG):
                    ch, hc = h // 2, h % 2
                    rows = slice(hc * 64, hc * 64 + 64)
                    pso = pspool.tile([128, QB], F32, name="ps_o", tag="po")
                    for kt_i in range(nkt):
                        diag = kt_i - qb * (QB // KT)
                        c0 = diag * KT if diag >= 0 else 0
                        pss = pspool.tile([128, QB], F32, name="ps_s", tag="ps")
                        nc.tensor.matmul(
                            pss[:, c0:QB],
                            kt_sb[rows, ch, kt_i * KT:(kt_i + 1) * KT],
                            qt_sb[rows, ch, qb * QB + c0:(qb + 1) * QB],
                            start=True,
                            stop=True,
                        )
                        e_sb = wpool.tile([128, QB], BF16, name="e_sb", tag="E", bufs=3)
                        nc.scalar.activation(
                            e_sb[:, c0:QB], pss[:, c0:QB],
                            mybir.ActivationFunctionType.Exp, scale=0.125,
                        )
                        if diag >= 0:
                            nc.vector.tensor_mul(
                                e_sb[:, c0:c0 + KT], e_sb[:, c0:c0 + KT], tri_sb[:]
                            )
                        nc.tensor.matmul(
                            pso[0:DKP, c0:QB],
                            v_aug[:, kt_i, h, :],
                            e_sb[:, c0:QB],
                            start=(kt_i == 0),
                            stop=(kt_i == nkt - 1),
                        )
                    u_av = wpool.tile([64, QB], BF16, name="u_av", tag="uav", bufs=4)
                    nc.vector.tensor_copy(u_av[:], pso[0:DK, 0:QB])
                    rden = wpool.tile([1, QB], F32, name="rden", tag="rden", bufs=4)
                    nc.vector.reciprocal(rden[0:1, :], pso[DK:DK + 1, 0:QB])
                    u_avs.append((u_av, rden))
                for h in range(HPG):
                    ch, hc = h // 2, h % 2
                    rows = slice(hc * 64, hc * 64 + 64)
                    u_av, rden = u_avs[h]
                    bc = wpool.tile([64, QB], F32, name="bc", tag="bc")
                    nc.gpsimd.partition_broadcast(bc[0:64, :], rden[0:1, :])
                    nc.vector.tensor_mul(
                        ot_sb[rows, ch, qb * QB:(qb + 1) * QB], u_av[:], bc[:]
                    )

                # partial output projection for this query block
                y_dram = dpool.tile([QB, D], BF16, name=f"y_dram_{qb}", tag=f"yd{qb}")
                for lt in range(QB // 128):
                    y_sb = wpool.tile([128, D], BF16, name="y_sb", tag="ysb")
                    for eh in range(2):
                        psy = pspool.tile([128, QB], F32, name="ps_y",
                                          tag="pp", bufs=3)
                        for ch in range(2):
                            nc.tensor.matmul(
                                psy[:],
                                ot_sb[:, ch, qb * QB + lt * 128:qb * QB + (lt + 1) * 128],
                                wo_sb[:, ch, eh * QB:(eh + 1) * QB],
                                start=(ch == 0),
                                stop=(ch == 1),
                            )
                        nc.vector.tensor_copy(y_sb[:, eh * QB:(eh + 1) * QB], psy[:])
                    nc.sync.dma_start(
                        out=y_dram[lt * 128:(lt + 1) * 128, :], in_=y_sb[:]
                    )
                if os.environ.get("KERNEL_NO_CC"):
                    # bisection mode: skip the collective, emit the local
                    # partial's first 128 rows (numerically wrong on purpose)
                    nc.sync.dma_start(
                        out=out_ext[qb * 128:(qb + 1) * 128, :],
                        in_=y_dram[0:128, :],
                    )
                else:
                    y_rs = dpool.tile([128, D], BF16, name=f"y_rs_{qb}", tag=f"yr{qb}")
                    nc.gpsimd.collective_compute(
                        "ReduceScatter",
                        mybir.AluOpType.add,
                        replica_groups=REPLICA_GROUPS,
                        ins=[y_dram[:]],
                        outs=[y_rs[:]],
                    )
                    nc.sync.dma_start(
                        out=out_ext[qb * 128:(qb + 1) * 128, :], in_=y_rs[:]
                    )

    nc.finalize()
    return nc


def _rope_tables(token_positions: np.ndarray) -> tuple[np.ndarray, np.ndarray]:
    """cos/sin lookup [128, L]: freq row j = r % 32, matching the per-head
    [E(32); O(32)] x 2-head chunk layout.  The sin table is sign-baked:
    +sin on E rows (read when producing O' = O*cos + E*sin), -sin on O rows
    (read when producing E' = E*cos - O*sin)."""
    j = np.arange(0, DK, 2, dtype=np.float32)  # 0,2,...,62
    freqs = (1.0 / (THETA ** (j / DK))).astype(np.float32)  # [32]
    pos = token_positions.astype(np.float32)  # [L]
    ang = pos[None, :] * freqs[:, None]  # [32, L] (f32 mul, matches reference)
    cos = np.cos(ang).astype(np.float32)
    sin = np.sin(ang).astype(np.float32)
    return np.tile(cos, (4, 1)), np.tile(np.vstack([sin, -sin]), (2, 1))


def _perm_rows(g: int) -> np.ndarray:
    """Q/K weight row permutation for head group g: per head, even dims then
    odd dims (cancels in the QK^T contraction; aligns rope to 32-row blocks)."""
    rows = []
    for hl in range(HPG):
        base = (g * HPG + hl) * DK
        rows.extend(base + np.arange(0, DK, 2))
        rows.extend(base + np.arange(1, DK, 2))
    return np.asarray(rows)


def _pack_w(w_t: np.ndarray) -> np.ndarray:
    """[D, DG] (in-dim major) -> [128, EC, DG] partition-major bf16."""
    return np.ascontiguousarray(
        w_t.reshape(EC, 128, DG).transpose(1, 0, 2).astype(ml_dtypes.bfloat16)
    )


_GRAPH_CACHE: list = []


def make_in_maps(inputs) -> list[dict]:
    x = np.asarray(inputs["x"], dtype=np.float32)
    token_positions = np.asarray(inputs["token_positions"])
    WQ = np.asarray(inputs["WQ"], dtype=np.float32)
    WK = np.asarray(inputs["WK"], dtype=np.float32)
    WV = np.asarray(inputs["WV"], dtype=np.float32)
    WO = np.asarray(inputs["WO"], dtype=np.float32)

    tri = np.ascontiguousarray(
        (np.arange(KT)[None, :] >= np.arange(KT)[:, None]).astype(ml_dtypes.bfloat16)
    )

    in_maps = []
    for c in range(8):
        b, g = c // 4, c % 4
        pr = _perm_rows(g)
        nrows = np.arange(g * DG, (g + 1) * DG)
        cos128, sin128 = _rope_tables(token_positions[b])
        # x: [L, D] -> [128, 4(quarter), EC, QB] partition-major
        xb = x[b].T.reshape(EC, 128, 4, QB).transpose(1, 2, 0, 3)
        wo_p = WO[:, nrows].T.reshape(2, 128, D).transpose(1, 0, 2)
        in_maps.append({
            "xh": np.ascontiguousarray(xb.astype(ml_dtypes.bfloat16)),
            "wq": _pack_w(WQ[pr, :].T),
            "wk": _pack_w(WK[pr, :].T),
            "wv": _pack_w(WV[nrows, :].T),
            "wo": np.ascontiguousarray(wo_p.astype(ml_dtypes.bfloat16)),
            "cs": np.ascontiguousarray(cos128),
            "sn": np.ascontiguousarray(sin128),
            "tri": tri,
        })
    return in_maps


def assemble(res: list[dict]) -> np.ndarray:
    out = np.empty((B, L, D), dtype=np.float32)
    for c in range(8):
        b, r = c // 4, c % 4
        yc = np.asarray(res[c]["out"]).astype(np.float32)  # [512, 1024]
        for qb in range(NQB):
            out[b, qb * QB + r * 128:qb * QB + (r + 1) * 128, :] = (
                yc[qb * 128:(qb + 1) * 128, :]
            )
    return out


def _install_ntff_hook():
    """The agent image lacks ``antenv.axon_hooks``; synthesize it and install
    the ctypes NTFF hook from trn_agent_boot so trace=True works."""
    import types
    import antenv
    if "antenv.axon_hooks" in sys.modules:
        return
    mod = types.ModuleType("antenv.axon_hooks")
    mod._hook = None
    mod.set_axon_ntff_profile_hook = lambda h: setattr(mod, "_hook", h)
    mod.get_axon_ntff_profile_hook = lambda: mod._hook
    sys.modules["antenv.axon_hooks"] = mod
    antenv.axon_hooks = mod
    try:
        from trn_agent_boot.trn_boot import _ntff_profile_via_ctypes
        mod._hook = _ntff_profile_via_ctypes("/opt/axon/libaxon_pjrt.so")
    except Exception as e:
        print(f"ntff hook install failed: {e}", file=sys.stderr)


def run_traced(in_maps):
    """Run with NTFF tracing; returns (results, BassKernelResults)."""
    _install_ntff_hook()
    if not _GRAPH_CACHE:
        _GRAPH_CACHE.append(build_graph())
    nc = _GRAPH_CACHE[0]
    os.environ["BASS_PERFETTO_PROFILE_ALL_CORES"] = "1"
    br = run_bass_kernel_spmd(nc, in_maps, core_ids=list(range(8)), trace=True)
    return br.results, br


def kernel(x, token_positions, WQ, WK, WV, WO):
    in_maps = make_in_maps(dict(
        x=x, token_positions=token_positions, WQ=WQ, WK=WK, WV=WV, WO=WO
    ))
    if not _GRAPH_CACHE:
        _GRAPH_CACHE.append(build_graph())
    nc = _GRAPH_CACHE[0]
    res = run_bass_kernel_spmd(nc, in_maps, core_ids=list(range(8))).results
    return assemble(res)


if __name__ == "__main__":
    rng = np.random.default_rng(0)
    ins = {
        "x": rng.standard_normal((B, L, D), dtype=np.float32),
        "token_positions": np.broadcast_to(np.arange(L, dtype=np.int32), (B, L)),
        "WQ": rng.standard_normal((D, D), dtype=np.float32) * 0.03,
        "WK": rng.standard_normal((D, D), dtype=np.float32) * 0.03,
        "WV": rng.standard_normal((D, D), dtype=np.float32) * 0.03,
        "WO": rng.standard_normal((D, D), dtype=np.float32) * 0.03,
    }
    y = kernel(**ins)
    print(y.shape, y.dtype, float(np.abs(y).mean()))
